# revision 3
# baseline (speedup 1.0000x reference)
"""Multi-head causal attention (B=2, S=2048, E=1024, H=16, D=64) on 8 TRN2
NeuronCores.

Sharding (data + tensor parallel, Megatron-style):
  core c -> batch b = c // 4, head group g = c % 4 (4 heads, e' = 256 cols).
  Wq/Wk/Wv column-sharded ([256, 1024] slices), Wo row-sharded
  ([1024, 256] slice); each core produces a partial output [2048, 1024]
  which the host sums per batch group (the Megatron all-reduce) and adds bo.

v2 pipeline (per-core, fp16 matmul operands, fp32 PSUM accumulate):
  - inputs DMA'd in 512-column chunks in consumption order so the first
    K-projection matmul starts ~4us in.
  - work split into 8 units (qt, c): q-tile 512 x head-pair chunk; per
    unit the k-loop runs lg (S^T tiles via PE quadrant-paired 64-row
    matmuls), exp on ACT (1/8 scale folded), triangular causal mask only
    on the 128-col diagonal sub-block, attn_v accumulation into a
    [65, 2, 512] PSUM acc whose row 64 is the softmax denominator.
    Diagonal tiles are column-narrowed (fully-masked columns never
    computed in lg/exp/attn_v).
  - normalize: DVE reciprocal of the denominator row ([1,512], PSUM),
    GpSimd partition-broadcast, one DVE multiply straight out of PSUM
    into valsT (no staging copy).
  - O-projection per 128-row chunk: [128,1024] PSUM, DVE copy to fp16,
    DMA out; emission interleaved two units behind attention so PE never
    waits on the normalize chain.  K/Q/V projection tiles for q-tile qt
    are emitted just before unit (qt, 0), spreading projection work
    through the attention phase.
  - PSUM budget: tag "lg" (4KB slots, bufs=2) serves proj/lg/O-proj
    tiles; tag "acc" (bufs=2) gives depth-2 unit pipelining.
"""
import sys
import os

sys.path.insert(0, "/opt/trn_rl_repo")

import numpy as np
from contextlib import ExitStack

import concourse.bass as bass  # noqa: E402
import concourse.mybir as mybir  # noqa: E402
import concourse.tile as tile  # noqa: E402
from concourse import bacc, bass_utils  # noqa: E402

bass_utils.upload_artifacts = lambda d: f"local:{d}"

B, S, E, H, D = 2, 2048, 1024, 16, 64
NCORES = 8
EL = 256  # e' columns per core (4 heads)
F32 = mybir.dt.float32
F16 = mybir.dt.float16
AF = mybir.ActivationFunctionType
NP16 = np.float16

_CACHE = {}


def _build():
    nc = bacc.Bacc("TRN2", target_bir_lowering=False, debug=False)

    xq_d = nc.dram_tensor("xqT", [E, S], F16, kind="ExternalInput")
    xk_d = nc.dram_tensor("xkT", [E, S], F16, kind="ExternalInput")
    xv_d = nc.dram_tensor("xvT", [E, S], F16, kind="ExternalInput")
    wq_d = nc.dram_tensor("wqT", [E, EL], F16, kind="ExternalInput")
    wk_d = nc.dram_tensor("wkT", [E, EL], F16, kind="ExternalInput")
    wv_d = nc.dram_tensor("wvT", [E, EL], F16, kind="ExternalInput")
    wo_d = nc.dram_tensor("woT", [EL, E], F16, kind="ExternalInput")
    bq_d = nc.dram_tensor("bq", [EL], F32, kind="ExternalInput")
    bk_d = nc.dram_tensor("bk", [EL], F32, kind="ExternalInput")
    bv_d = nc.dram_tensor("bv", [EL], F32, kind="ExternalInput")
    vones_d = nc.dram_tensor("vones", [128, 16, 4, 1], F16, kind="ExternalInput")
    tri_d = nc.dram_tensor("tri", [128, 128], F16, kind="ExternalInput")
    out_d = nc.dram_tensor("out", [S, E], F16, kind="ExternalOutput")

    with tile.TileContext(nc) as tc, ExitStack() as ctx:
        cpool = ctx.enter_context(tc.tile_pool(name="const", bufs=1))
        psp = ctx.enter_context(tc.tile_pool(name="psp", bufs=2, space="PSUM"))
        expp = ctx.enter_context(tc.tile_pool(name="expp", bufs=6))
        smp = ctx.enter_context(tc.tile_pool(name="smp", bufs=2))
        otp = ctx.enter_context(tc.tile_pool(name="otp", bufs=2))

        def chunk(dst, src_d, i):
            nc.sync.dma_start(
                dst[:, :, i * 512:(i + 1) * 512],
                src_d.ap().rearrange("(k p) m -> p k m", p=128)
                [:, :, i * 512:(i + 1) * 512])

        # ---- constants + inputs, in consumption order ----
        wk = cpool.tile([128, 8, EL], F16, tag="wk")
        nc.sync.dma_start(wk[:], wk_d.ap().rearrange("(k p) m -> p k m", p=128))
        bkt = cpool.tile([128, 2], F32, tag="bkt")
        nc.sync.dma_start(bkt[:], bk_d.ap().rearrange("(c p) -> p c", p=128))
        xk = cpool.tile([128, 8, S], F16, tag="xk")
        xq = cpool.tile([128, 8, S], F16, tag="xq")
        xv = cpool.tile([128, 8, S], F16, tag="xv")
        chunk(xk, xk_d, 0)

        wq = cpool.tile([128, 8, EL], F16, tag="wq")
        nc.sync.dma_start(wq[:], wq_d.ap().rearrange("(k p) m -> p k m", p=128))
        bqt = cpool.tile([128, 2], F32, tag="bqt")
        nc.sync.dma_start(bqt[:], bq_d.ap().rearrange("(c p) -> p c", p=128))
        chunk(xq, xq_d, 0)

        wv = cpool.tile([128, 8, EL], F16, tag="wv")
        nc.sync.dma_start(wv[:], wv_d.ap().rearrange("(k p) m -> p k m", p=128))
        bvr = cpool.tile([1, EL], F32, tag="bvr")
        nc.sync.dma_start(bvr[:], bv_d.ap().rearrange("(p m) -> p m", p=1))
        bvb = cpool.tile([128, EL], F32, tag="bvb")
        nc.gpsimd.partition_broadcast(bvb[:], bvr[:])
        chunk(xv, xv_d, 0)

        tri = cpool.tile([128, 128], F16, tag="tri")
        nc.sync.dma_start(tri[:], tri_d.ap())
        wo = cpool.tile([128, 2, E], F16, tag="wo")
        nc.sync.dma_start(wo[:], wo_d.ap().rearrange("(c p) m -> p c m", p=128))

        KT = cpool.tile([128, 2, S], F16, tag="KT")
        QT = cpool.tile([128, 2, S], F16, tag="QT")
        VP = cpool.tile([128, 16, 4 * 66], F16, tag="VP")  # 66: 4B-aligned
        valsT = cpool.tile([128, 2, S], F16, tag="valsT")

        # ones columns of V' (col 64 of each 66-block)
        nc.sync.dma_start(
            VP[:].rearrange("p k (h x) -> p k h x", h=4)[:, :, :, 64:65],
            vones_d.ap(),
        )

        # remaining input chunks (arrive while early projections run)
        for i in range(1, 4):
            chunk(xk, xk_d, i)
            chunk(xq, xq_d, i)
            chunk(xv, xv_d, i)

        # ---- projection helpers ----
        def proj_kq(x_t, w_t, b_t, out_t, tt, c, pfx):
            ps = psp.tile([128, 512], F32, tag="lg", name=f"{pfx}ps{tt}_{c}")
            for k in range(8):
                nc.tensor.matmul(
                    ps[:],
                    lhsT=w_t[:, k, c * 128:(c + 1) * 128],
                    rhs=x_t[:, k, tt * 512:(tt + 1) * 512],
                    start=(k == 0), stop=(k == 7))
            nc.vector.tensor_scalar_add(
                out_t[:, c, tt * 512:(tt + 1) * 512], ps[:], b_t[:, c:c + 1])

        def proj_v(t3):
            ps = psp.tile([128, EL], F32, tag="lg", name=f"vps{t3}")
            for k in range(8):
                nc.tensor.matmul(
                    ps[:],
                    lhsT=xv[:, k, t3 * 128:(t3 + 1) * 128],
                    rhs=wv[:, k, :],
                    start=(k == 0), stop=(k == 7))
            nc.vector.tensor_add(
                VP[:, t3, :].rearrange("p (h x) -> p h x", h=4)[:, :, 0:64],
                ps[:].rearrange("p (h x) -> p h x", h=4),
                bvb[:].rearrange("p (h x) -> p h x", h=4))

        # ---- O-projection for one 128-row output chunk ----
        def oproj_tt(tt):
            ops = psp.tile([128, 2, 512], F32, tag="lg", name=f"ops{tt}")
            for eo in range(2):
                for c in range(2):
                    nc.tensor.matmul(
                        ops[:, eo, :],
                        lhsT=valsT[:, c, tt * 128:(tt + 1) * 128],
                        rhs=wo[:, c, eo * 512:(eo + 1) * 512],
                        start=(c == 0), stop=(c == 1))
            ot = otp.tile([128, 2, 512], F16, tag="ot", name=f"ot{tt}")
            nc.vector.tensor_copy(ot[:], ops[:])
            nc.sync.dma_start(
                out_d.ap()[tt * 128:(tt + 1) * 128, :]
                .rearrange("p (a b) -> p a b", a=2), ot[:])

        # ---- one attention unit: (q-tile qt) x (head-pair chunk c) ----
        def unit(qt, c, opq):
            nkt = 4 * qt + 4
            acc = psp.tile([65, 2, 512], F32, tag="acc", name=f"acc{qt}_{c}")
            exs = {}

            def lg_exp(kt):
                dd = max(0, kt * 128 - qt * 512)
                lg = psp.tile([128, 2, 512], F32, tag="lg",
                              name=f"lg{qt}_{c}_{kt}")
                for hh in range(2):
                    nc.tensor.matmul(
                        lg[:, hh, dd:512],
                        lhsT=KT[hh * 64:(hh + 1) * 64, c,
                                kt * 128:(kt + 1) * 128],
                        rhs=QT[hh * 64:(hh + 1) * 64, c,
                               qt * 512 + dd:(qt + 1) * 512],
                        start=True, stop=True)
                ex = expp.tile([128, 2, 512], F16, tag="ex",
                               name=f"ex{qt}_{c}_{kt}")
                nc.scalar.activation(ex[:, :, dd:512], lg[:, :, dd:512],
                                     AF.Exp, scale=0.125)
                if kt * 128 >= qt * 512:  # diagonal: triangular 128-col mask
                    for hh in range(2):
                        nc.vector.tensor_mul(ex[:, hh, dd:dd + 128],
                                             ex[:, hh, dd:dd + 128], tri[:])
                exs[kt] = (ex, dd)

            def attn_v(kt):
                ex, dd = exs.pop(kt)
                for hh in range(2):
                    h = 2 * c + hh
                    nc.tensor.matmul(
                        acc[:, hh, dd:512],
                        lhsT=VP[:, kt, h * 66:h * 66 + 65],
                        rhs=ex[:, hh, dd:512],
                        start=(kt == 0), stop=(kt == nkt - 1),
                        skip_group_check=True)

            for kt in range(nkt):
                lg_exp(kt)
                if kt >= 2:
                    attn_v(kt - 2)
                if kt in (3, 7) and opq:
                    oproj_tt(opq.pop(0))
            attn_v(max(nkt - 2, 0))
            if nkt >= 2:
                attn_v(nkt - 1)
            while opq:
                oproj_tt(opq.pop(0))

            # normalize straight out of PSUM; runs on DVE/GpSimd while the
            # next unit's attention streams on PE/ACT
            for hh in range(2):
                rr = smp.tile([1, 512], F32, tag="rr",
                              name=f"rr{qt}_{c}_{hh}")
                nc.vector.reciprocal(rr[:], acc[64:65, hh, :])
                bcS = smp.tile([64, 512], F32, tag="bc",
                               name=f"bc{qt}_{c}_{hh}")
                nc.gpsimd.partition_broadcast(bcS[:], rr[:])
                nc.vector.tensor_mul(
                    valsT[hh * 64:(hh + 1) * 64, c,
                          qt * 512:(qt + 1) * 512],
                    acc[0:64, hh, :], bcS[:])

        # ---- schedule ----
        for qt in range(4):
            for cc in range(2):
                proj_kq(xk, wk, bkt, KT, qt, cc, "k")
            for cc in range(2):
                proj_kq(xq, wq, bqt, QT, qt, cc, "q")
            for t3 in range(4 * qt, 4 * qt + 4):
                proj_v(t3)
            for cc in range(2):
                opq = ([4 * (qt - 1) + 2 * cc, 4 * (qt - 1) + 2 * cc + 1]
                       if qt >= 1 else [])
                unit(qt, cc, opq)
        for tt in range(12, 16):
            oproj_tt(tt)

    nc.compile()
    return nc


def get_nc():
    if "nc" not in _CACHE:
        _CACHE["nc"] = _build()
    return _CACHE["nc"]


def _tri():
    i = np.arange(128)[:, None]
    j = np.arange(128)[None, :]
    return (i <= j).astype(NP16)


def make_in_maps(query, key, value, Wq, bq, Wk, bk, Wv, bv, Wo, bo):
    query = np.asarray(query, np.float32)
    key = np.asarray(key, np.float32)
    value = np.asarray(value, np.float32)
    Wq, Wk, Wv, Wo = (np.asarray(a, np.float32) for a in (Wq, Wk, Wv, Wo))
    bq, bk, bv = (np.asarray(a, np.float32) for a in (bq, bk, bv))
    tri = _tri()
    vones = np.ones((128, 16, 4, 1), NP16)
    in_maps = []
    for c in range(NCORES):
        b, g = divmod(c, 4)
        sl = slice(g * EL, (g + 1) * EL)
        in_maps.append({
            "xqT": np.ascontiguousarray(query[b].T).astype(NP16),
            "xkT": np.ascontiguousarray(key[b].T).astype(NP16),
            "xvT": np.ascontiguousarray(value[b].T).astype(NP16),
            "wqT": np.ascontiguousarray(Wq[sl, :].T).astype(NP16),
            "wkT": np.ascontiguousarray(Wk[sl, :].T).astype(NP16),
            "wvT": np.ascontiguousarray(Wv[sl, :].T).astype(NP16),
            "woT": np.ascontiguousarray(Wo[:, sl].T).astype(NP16),
            "bq": np.ascontiguousarray(bq[sl]),
            "bk": np.ascontiguousarray(bk[sl]),
            "bv": np.ascontiguousarray(bv[sl]),
            "vones": vones,
            "tri": tri,
        })
    return in_maps


def run(inputs, trace=False, tmpdir=None):
    """Run on 8 cores; returns (full_output, BassKernelResults)."""
    nc = get_nc()
    in_maps = make_in_maps(**inputs)
    res = bass_utils.run_bass_kernel_spmd(
        nc, in_maps, list(range(NCORES)), trace=trace, tmpdir=tmpdir)
    bo = np.asarray(inputs["bo"], np.float32)
    out = np.zeros((B, S, E), np.float32)
    for c in range(NCORES):
        out[c // 4] += res.results[c]["out"].astype(np.float32)
    out += bo[None, None, :]
    return out, res


def kernel(**inputs):
    out, _ = run(inputs)
    return out


# revision 10
# speedup vs baseline: 1.3190x; 1.3190x over previous
"""Multi-head causal attention (B=2, S=2048, E=1024, H=16, D=64) on 8 TRN2
NeuronCores.

Sharding (data + tensor parallel, Megatron-style):
  core c -> batch b = c // 4, head group g = c % 4 (4 heads, e' = 256 cols).
  Wq/Wk/Wv column-sharded ([256, 1024] slices), Wo row-sharded
  ([1024, 256] slice); each core produces a partial output [2048, 1024]
  which the host sums per batch group (the Megatron all-reduce) and adds bo.

v2 pipeline (per-core, fp16 matmul operands, fp32 PSUM accumulate):
  - inputs DMA'd in 512-column chunks in consumption order so the first
    K-projection matmul starts ~4us in.
  - work split into 8 units (qt, c): q-tile 512 x head-pair chunk; per
    unit the k-loop runs lg (S^T tiles via PE quadrant-paired 64-row
    matmuls), exp on ACT (1/8 scale folded), triangular causal mask only
    on the 128-col diagonal sub-block, attn_v accumulation into a
    [65, 2, 512] PSUM acc whose row 64 is the softmax denominator.
    Diagonal tiles are column-narrowed (fully-masked columns never
    computed in lg/exp/attn_v).
  - normalize: DVE reciprocal of the denominator row ([1,512], PSUM),
    GpSimd partition-broadcast, one DVE multiply straight out of PSUM
    into valsT (no staging copy).
  - O-projection per 128-row chunk: [128,1024] PSUM, DVE copy to fp16,
    DMA out; emission interleaved two units behind attention so PE never
    waits on the normalize chain.  K/Q/V projection tiles for q-tile qt
    are emitted just before unit (qt, 0), spreading projection work
    through the attention phase.
  - PSUM budget: tag "lg" (4KB slots, bufs=2) serves proj/lg/O-proj
    tiles; tag "acc" (bufs=2) gives depth-2 unit pipelining.
"""
import sys
import os

sys.path.insert(0, "/opt/trn_rl_repo")

import numpy as np
from contextlib import ExitStack

import concourse.bass as bass  # noqa: E402
import concourse.mybir as mybir  # noqa: E402
import concourse.tile as tile  # noqa: E402
from concourse import bacc, bass_utils  # noqa: E402

bass_utils.upload_artifacts = lambda d: f"local:{d}"

B, S, E, H, D = 2, 2048, 1024, 16, 64
NCORES = 8
EL = 256  # e' columns per core (4 heads)
F32 = mybir.dt.float32
F16 = mybir.dt.float16
AF = mybir.ActivationFunctionType
NP16 = np.float16

_CACHE = {}


def _build():
    nc = bacc.Bacc("TRN2", target_bir_lowering=False, debug=False)

    # inputs pre-laid-out host-side for contiguous 8KB DMA descriptors:
    # x tensors [chunk, partition, k, 512], weights [partition, k, cols]
    xq_d = nc.dram_tensor("xqT", [4, 128, 8, 512], F16, kind="ExternalInput")
    xk_d = nc.dram_tensor("xkT", [4, 128, 8, 512], F16, kind="ExternalInput")
    xv_d = nc.dram_tensor("xvT", [4, 128, 8, 512], F16, kind="ExternalInput")
    wq_d = nc.dram_tensor("wqT", [128, 8, EL], F16, kind="ExternalInput")
    wk_d = nc.dram_tensor("wkT", [128, 8, EL], F16, kind="ExternalInput")
    wv_d = nc.dram_tensor("wvT", [128, 8, EL], F16, kind="ExternalInput")
    wo_d = nc.dram_tensor("woT", [128, 2, E], F16, kind="ExternalInput")
    bq_d = nc.dram_tensor("bq", [EL], F32, kind="ExternalInput")
    bk_d = nc.dram_tensor("bk", [EL], F32, kind="ExternalInput")
    bv_d = nc.dram_tensor("bv", [EL], F32, kind="ExternalInput")
    vones_d = nc.dram_tensor("vones", [128, 16, 4, 1], F16, kind="ExternalInput")
    tri_d = nc.dram_tensor("tri", [128, 128], F16, kind="ExternalInput")
    out_d = nc.dram_tensor("out", [S, E], F16, kind="ExternalOutput")

    with tile.TileContext(nc) as tc, ExitStack() as ctx:
        cpool = ctx.enter_context(tc.tile_pool(name="const", bufs=1))
        psp = ctx.enter_context(tc.tile_pool(name="psp", bufs=2, space="PSUM"))
        expp = ctx.enter_context(tc.tile_pool(name="expp", bufs=6))
        smp = ctx.enter_context(tc.tile_pool(name="smp", bufs=2))
        otp = ctx.enter_context(tc.tile_pool(name="otp", bufs=2))

        def chunk(dst, src_d, i):
            nc.sync.dma_start(dst[:, :, i * 512:(i + 1) * 512],
                              src_d.ap()[i])

        # ---- constants + inputs, in consumption order ----
        wk = cpool.tile([128, 8, EL], F16, tag="wk")
        nc.sync.dma_start(wk[:], wk_d.ap())
        xk = cpool.tile([128, 8, S], F16, tag="xk")
        xq = cpool.tile([128, 8, S], F16, tag="xq")
        xv = cpool.tile([128, 8, S], F16, tag="xv")
        chunk(xk, xk_d, 0)
        bkt = cpool.tile([128, 2], F32, tag="bkt")
        nc.sync.dma_start(bkt[:], bk_d.ap().rearrange("(c p) -> p c", p=128))

        wq = cpool.tile([128, 8, EL], F16, tag="wq")
        nc.sync.dma_start(wq[:], wq_d.ap())
        chunk(xq, xq_d, 0)
        bqt = cpool.tile([128, 2], F32, tag="bqt")
        nc.sync.dma_start(bqt[:], bq_d.ap().rearrange("(c p) -> p c", p=128))

        wv = cpool.tile([128, 8, EL], F16, tag="wv")
        nc.sync.dma_start(wv[:], wv_d.ap())
        chunk(xv, xv_d, 0)
        bvr = cpool.tile([1, EL], F32, tag="bvr")
        nc.sync.dma_start(bvr[:], bv_d.ap().rearrange("(p m) -> p m", p=1))
        bvb = cpool.tile([128, EL], F32, tag="bvb")
        nc.gpsimd.partition_broadcast(bvb[:], bvr[:])

        tri = cpool.tile([128, 128], F16, tag="tri")
        nc.sync.dma_start(tri[:], tri_d.ap())
        wo = cpool.tile([128, 2, E], F16, tag="wo")
        nc.sync.dma_start(wo[:], wo_d.ap())

        KT = cpool.tile([128, 2, S], F16, tag="KT")
        QT = cpool.tile([128, 2, S], F16, tag="QT")
        VP = cpool.tile([128, 16, 4 * 66], F16, tag="VP")  # 66: 4B-aligned
        valsT = cpool.tile([128, 2, S], F16, tag="valsT")

        # ones columns of V' (col 64 of each 66-block)
        nc.sync.dma_start(
            VP[:].rearrange("p k (h x) -> p k h x", h=4)[:, :, :, 64:65],
            vones_d.ap(),
        )

        # remaining input chunks (arrive while early projections run)
        for i in range(1, 4):
            chunk(xk, xk_d, i)
            chunk(xq, xq_d, i)
            chunk(xv, xv_d, i)

        # ---- projection helpers ----
        def proj_kq(x_t, w_t, b_t, out_t, tt, c, pfx):
            ps = psp.tile([128, 512], F32, tag="lg", name=f"{pfx}ps{tt}_{c}")
            for k in range(8):
                nc.tensor.matmul(
                    ps[:],
                    lhsT=w_t[:, k, c * 128:(c + 1) * 128],
                    rhs=x_t[:, k, tt * 512:(tt + 1) * 512],
                    start=(k == 0), stop=(k == 7))
            nc.vector.tensor_scalar_add(
                out_t[:, c, tt * 512:(tt + 1) * 512], ps[:], b_t[:, c:c + 1])

        def proj_v(t3):
            ps = psp.tile([128, EL], F32, tag="lg", name=f"vps{t3}")
            for k in range(8):
                nc.tensor.matmul(
                    ps[:],
                    lhsT=xv[:, k, t3 * 128:(t3 + 1) * 128],
                    rhs=wv[:, k, :],
                    start=(k == 0), stop=(k == 7))
            nc.vector.tensor_add(
                VP[:, t3, :].rearrange("p (h x) -> p h x", h=4)[:, :, 0:64],
                ps[:].rearrange("p (h x) -> p h x", h=4),
                bvb[:].rearrange("p (h x) -> p h x", h=4))

        # ---- O-projection for one 128-row output chunk ----
        def oproj_tt(tt):
            ops = psp.tile([128, 2, 512], F32, tag="lg", name=f"ops{tt}")
            for eo in range(2):
                for c in range(2):
                    nc.tensor.matmul(
                        ops[:, eo, :],
                        lhsT=valsT[:, c, tt * 128:(tt + 1) * 128],
                        rhs=wo[:, c, eo * 512:(eo + 1) * 512],
                        start=(c == 0), stop=(c == 1))
            ot = otp.tile([128, 2, 512], F16, tag="ot", name=f"ot{tt}")
            nc.vector.tensor_copy(ot[:], ops[:])
            nc.sync.dma_start(
                out_d.ap()[tt * 128:(tt + 1) * 128, :]
                .rearrange("p (a b) -> p a b", a=2), ot[:])

        # ---- one attention unit: (q-tile qt) x (head-pair chunk c) ----
        def unit(qt, c, opq):
            nkt = 4 * qt + 4
            acc = psp.tile([65, 2, 512], F32, tag="acc", name=f"acc{qt}_{c}")
            exs = {}

            def lg_exp(kt):
                dd = max(0, kt * 128 - qt * 512)
                lg = psp.tile([128, 2, 512], F32, tag="lg",
                              name=f"lg{qt}_{c}_{kt}")
                for hh in range(2):
                    nc.tensor.matmul(
                        lg[:, hh, :],
                        lhsT=KT[hh * 64:(hh + 1) * 64, c,
                                kt * 128:(kt + 1) * 128],
                        rhs=QT[hh * 64:(hh + 1) * 64, c,
                               qt * 512:(qt + 1) * 512],
                        start=True, stop=True)
                ex = expp.tile([128, 2, 512], F16, tag="ex",
                               name=f"ex{qt}_{c}_{kt}")
                # full-width: per-call ACT overhead eats any narrowing gain
                nc.scalar.activation(ex[:, :, :], lg[:, :, :],
                                     AF.Exp, scale=0.125)
                if kt * 128 >= qt * 512:  # diagonal: triangular 128-col mask
                    for hh in range(2):
                        nc.vector.tensor_mul(ex[:, hh, dd:dd + 128],
                                             ex[:, hh, dd:dd + 128], tri[:])
                exs[kt] = (ex, dd)

            def attn_v(kt):
                ex, dd = exs.pop(kt)
                for hh in range(2):
                    h = 2 * c + hh
                    nc.tensor.matmul(
                        acc[:, hh, dd:512],
                        lhsT=VP[:, kt, h * 66:h * 66 + 65],
                        rhs=ex[:, hh, dd:512],
                        start=(kt == 0), stop=(kt == nkt - 1),
                        skip_group_check=True)

            for kt in range(nkt):
                lg_exp(kt)
                if kt >= 2:
                    attn_v(kt - 2)
                if kt in (3, 7) and opq:
                    oproj_tt(opq.pop(0))
            attn_v(max(nkt - 2, 0))
            if nkt >= 2:
                attn_v(nkt - 1)
            while opq:
                oproj_tt(opq.pop(0))

            # normalize straight out of PSUM; runs on DVE/GpSimd while the
            # next unit's attention streams on PE/ACT
            # reciprocal on a DMA-transposed [128, 8] column layout
            # (a [1, 1024] single-lane reciprocal would cost ~8us on DVE);
            # DMA kicks ride the idle GpSimd DGE
            denS = smp.tile([1, 1024], F32, tag="denS",
                            name=f"denS{qt}_{c}")
            nc.vector.tensor_copy(
                denS[:], acc[64:65, :, :].rearrange("p a b -> p (a b)"))
            lcol = smp.tile([128, 8], F32, tag="lcol", name=f"lcol{qt}_{c}")
            nc.gpsimd.dma_start(
                lcol[:, :],
                denS[0:1, :].rearrange("p (a b) -> p a b", a=128))
            rcol = smp.tile([128, 8], F32, tag="rcol", name=f"rcol{qt}_{c}")
            nc.vector.reciprocal(rcol[:, :], lcol[:, :])
            rr = smp.tile([1, 1024], F32, tag="rr", name=f"rr{qt}_{c}")
            nc.gpsimd.dma_start(
                rr[0:1, :].rearrange("p (a b) -> p a b", a=128),
                rcol[:, :])
            bcS = smp.tile([64, 1024], F32, tag="bc", name=f"bc{qt}_{c}")
            nc.gpsimd.partition_broadcast(bcS[:], rr[:])
            for hh in range(2):
                nc.vector.tensor_mul(
                    valsT[hh * 64:(hh + 1) * 64, c,
                          qt * 512:(qt + 1) * 512],
                    acc[0:64, hh, :],
                    bcS[:, hh * 512:(hh + 1) * 512])

        # ---- schedule ----
        for qt in range(4):
            for cc in range(2):
                proj_kq(xk, wk, bkt, KT, qt, cc, "k")
            for cc in range(2):
                proj_kq(xq, wq, bqt, QT, qt, cc, "q")
            for t3 in range(4 * qt, 4 * qt + 4):
                proj_v(t3)
            for cc in range(2):
                opq = ([4 * (qt - 1) + 2 * cc, 4 * (qt - 1) + 2 * cc + 1]
                       if qt >= 1 else [])
                unit(qt, cc, opq)
        for tt in range(12, 16):
            oproj_tt(tt)

    nc.compile()
    return nc


def get_nc():
    if "nc" not in _CACHE:
        _CACHE["nc"] = _build()
    return _CACHE["nc"]


def _tri():
    i = np.arange(128)[:, None]
    j = np.arange(128)[None, :]
    return (i <= j).astype(NP16)


def make_in_maps(query, key, value, Wq, bq, Wk, bk, Wv, bv, Wo, bo):
    query = np.asarray(query, np.float32)
    key = np.asarray(key, np.float32)
    value = np.asarray(value, np.float32)
    Wq, Wk, Wv, Wo = (np.asarray(a, np.float32) for a in (Wq, Wk, Wv, Wo))
    bq, bk, bv = (np.asarray(a, np.float32) for a in (bq, bk, bv))
    tri = _tri()
    vones = np.ones((128, 16, 4, 1), NP16)

    def xlay(x):
        # x [S, E] -> x.T [E, S] -> [chunk 4, partition 128, k 8, 512]
        return np.ascontiguousarray(
            x.T.reshape(8, 128, 4, 512).transpose(2, 1, 0, 3)).astype(NP16)

    def wlay(w):
        # w [e', E] -> w.T [E, e'] -> [partition 128, k 8, e']
        return np.ascontiguousarray(
            w.T.reshape(8, 128, -1).transpose(1, 0, 2)).astype(NP16)

    in_maps = []
    for c in range(NCORES):
        b, g = divmod(c, 4)
        sl = slice(g * EL, (g + 1) * EL)
        in_maps.append({
            "xqT": xlay(query[b]),
            "xkT": xlay(key[b]),
            "xvT": xlay(value[b]),
            "wqT": wlay(Wq[sl, :]),
            "wkT": wlay(Wk[sl, :]),
            "wvT": wlay(Wv[sl, :]),
            "woT": np.ascontiguousarray(
                Wo[:, sl].T.reshape(2, 128, E).transpose(1, 0, 2)
            ).astype(NP16),
            "bq": np.ascontiguousarray(bq[sl]),
            "bk": np.ascontiguousarray(bk[sl]),
            "bv": np.ascontiguousarray(bv[sl]),
            "vones": vones,
            "tri": tri,
        })
    return in_maps


def run(inputs, trace=False, tmpdir=None):
    """Run on 8 cores; returns (full_output, BassKernelResults)."""
    nc = get_nc()
    in_maps = make_in_maps(**inputs)
    res = bass_utils.run_bass_kernel_spmd(
        nc, in_maps, list(range(NCORES)), trace=trace, tmpdir=tmpdir)
    bo = np.asarray(inputs["bo"], np.float32)
    out = np.zeros((B, S, E), np.float32)
    for c in range(NCORES):
        out[c // 4] += res.results[c]["out"].astype(np.float32)
    out += bo[None, None, :]
    return out, res


def kernel(**inputs):
    out, _ = run(inputs)
    return out


# revision 15
# speedup vs baseline: 1.3963x; 1.0586x over previous
"""Multi-head causal attention (B=2, S=2048, E=1024, H=16, D=64) on 8 TRN2
NeuronCores.

Sharding (data + tensor parallel, Megatron-style):
  core c -> batch b = c // 4, head group g = c % 4 (4 heads, e' = 256 cols).
  Wq/Wk/Wv column-sharded ([256, 1024] slices), Wo row-sharded
  ([1024, 256] slice); each core produces a partial output [2048, 1024]
  which the host sums per batch group (the Megatron all-reduce) and adds bo.

v2 pipeline (per-core, fp16 matmul operands, fp32 PSUM accumulate):
  - inputs DMA'd in 512-column chunks in consumption order so the first
    K-projection matmul starts ~4us in.
  - work split into 8 units (qt, c): q-tile 512 x head-pair chunk; per
    unit the k-loop runs lg (S^T tiles via PE quadrant-paired 64-row
    matmuls), exp on ACT (1/8 scale folded), triangular causal mask only
    on the 128-col diagonal sub-block, attn_v accumulation into a
    [65, 2, 512] PSUM acc whose row 64 is the softmax denominator.
    Diagonal tiles are column-narrowed (fully-masked columns never
    computed in lg/exp/attn_v).
  - normalize: DVE reciprocal of the denominator row ([1,512], PSUM),
    GpSimd partition-broadcast, one DVE multiply straight out of PSUM
    into valsT (no staging copy).
  - O-projection per 128-row chunk: [128,1024] PSUM, DVE copy to fp16,
    DMA out; emission interleaved two units behind attention so PE never
    waits on the normalize chain.  K/Q/V projection tiles for q-tile qt
    are emitted just before unit (qt, 0), spreading projection work
    through the attention phase.
  - PSUM budget: tag "lg" (4KB slots, bufs=2) serves proj/lg/O-proj
    tiles; tag "acc" (bufs=2) gives depth-2 unit pipelining.
"""
import sys
import os

sys.path.insert(0, "/opt/trn_rl_repo")

import numpy as np
from contextlib import ExitStack

import concourse.bass as bass  # noqa: E402
import concourse.mybir as mybir  # noqa: E402
import concourse.tile as tile  # noqa: E402
from concourse import bacc, bass_utils  # noqa: E402

bass_utils.upload_artifacts = lambda d: f"local:{d}"

B, S, E, H, D = 2, 2048, 1024, 16, 64
NCORES = 8
EL = 256  # e' columns per core (4 heads)
F32 = mybir.dt.float32
F16 = mybir.dt.float16
AF = mybir.ActivationFunctionType
NP16 = np.float16

_CACHE = {}


def _build():
    nc = bacc.Bacc("TRN2", target_bir_lowering=False, debug=False)

    # inputs pre-laid-out host-side for contiguous 8KB DMA descriptors:
    # x tensors [chunk, partition, k, 512], weights [partition, k, cols]
    xq_d = nc.dram_tensor("xqT", [4, 128, 8, 512], F16, kind="ExternalInput")
    xk_d = nc.dram_tensor("xkT", [4, 128, 8, 512], F16, kind="ExternalInput")
    xv_d = nc.dram_tensor("xvT", [4, 128, 8, 512], F16, kind="ExternalInput")
    wq_d = nc.dram_tensor("wqT", [128, 8, EL], F16, kind="ExternalInput")
    wk_d = nc.dram_tensor("wkT", [128, 8, EL], F16, kind="ExternalInput")
    wv_d = nc.dram_tensor("wvT", [128, 8, EL], F16, kind="ExternalInput")
    wo_d = nc.dram_tensor("woT", [128, 2, E], F16, kind="ExternalInput")
    bq_d = nc.dram_tensor("bq", [EL], F32, kind="ExternalInput")
    bk_d = nc.dram_tensor("bk", [EL], F32, kind="ExternalInput")
    bv_d = nc.dram_tensor("bv", [EL], F32, kind="ExternalInput")
    vones_d = nc.dram_tensor("vones", [128, 16, 4, 1], F16, kind="ExternalInput")
    tri_d = nc.dram_tensor("tri", [128, 128], F16, kind="ExternalInput")
    out_d = nc.dram_tensor("out", [S, E], F16, kind="ExternalOutput")

    with tile.TileContext(nc) as tc, ExitStack() as ctx:
        cpool = ctx.enter_context(tc.tile_pool(name="const", bufs=1))
        psp = ctx.enter_context(tc.tile_pool(name="psp", bufs=2, space="PSUM"))
        expp = ctx.enter_context(tc.tile_pool(name="expp", bufs=6))
        smp = ctx.enter_context(tc.tile_pool(name="smp", bufs=2))
        otp = ctx.enter_context(tc.tile_pool(name="otp", bufs=2))

        def chunk(dst, src_d, i):
            nc.sync.dma_start(dst[:, :, i * 512:(i + 1) * 512],
                              src_d.ap()[i])

        # ---- constants + inputs, in consumption order ----
        wk = cpool.tile([128, 8, EL], F16, tag="wk")
        nc.sync.dma_start(wk[:], wk_d.ap())
        xk = cpool.tile([128, 8, S], F16, tag="xk")
        xq = cpool.tile([128, 8, S], F16, tag="xq")
        xv = cpool.tile([128, 8, S], F16, tag="xv")
        chunk(xk, xk_d, 0)
        bkt = cpool.tile([128, 2], F32, tag="bkt")
        nc.sync.dma_start(bkt[:], bk_d.ap().rearrange("(c p) -> p c", p=128))

        wq = cpool.tile([128, 8, EL], F16, tag="wq")
        nc.sync.dma_start(wq[:], wq_d.ap())
        chunk(xq, xq_d, 0)
        bqt = cpool.tile([128, 2], F32, tag="bqt")
        nc.sync.dma_start(bqt[:], bq_d.ap().rearrange("(c p) -> p c", p=128))

        wv = cpool.tile([128, 8, EL], F16, tag="wv")
        nc.sync.dma_start(wv[:], wv_d.ap())
        chunk(xv, xv_d, 0)
        bvr = cpool.tile([1, EL], F32, tag="bvr")
        nc.sync.dma_start(bvr[:], bv_d.ap().rearrange("(p m) -> p m", p=1))
        bvb = cpool.tile([128, EL], F32, tag="bvb")
        nc.gpsimd.partition_broadcast(bvb[:], bvr[:])

        tri = cpool.tile([128, 128], F16, tag="tri")
        nc.sync.dma_start(tri[:], tri_d.ap())
        wo = cpool.tile([128, 2, E], F16, tag="wo")

        KT = cpool.tile([128, 2, S], F16, tag="KT")
        QT = cpool.tile([128, 2, S], F16, tag="QT")
        VP = cpool.tile([128, 16, 4 * 66], F16, tag="VP")  # 66: 4B-aligned
        valsT = cpool.tile([128, 2, S], F16, tag="valsT")

        # ones columns of V' (col 64 of each 66-block)
        nc.sync.dma_start(
            VP[:].rearrange("p k (h x) -> p k h x", h=4)[:, :, :, 64:65],
            vones_d.ap(),
        )

        # remaining input chunks (arrive while early projections run);
        # wo rides after chunk 1 (first needed at O-proj of qt=0, ~45us)
        for i in range(1, 4):
            chunk(xk, xk_d, i)
            chunk(xq, xq_d, i)
            chunk(xv, xv_d, i)
            if i == 1:
                nc.sync.dma_start(wo[:], wo_d.ap())

        # ---- projection helpers ----
        def proj_kq(x_t, w_t, b_t, out_t, tt, c, pfx):
            ps = psp.tile([128, 512], F32, tag="lg", name=f"{pfx}ps{tt}_{c}")
            for k in range(8):
                nc.tensor.matmul(
                    ps[:],
                    lhsT=w_t[:, k, c * 128:(c + 1) * 128],
                    rhs=x_t[:, k, tt * 512:(tt + 1) * 512],
                    start=(k == 0), stop=(k == 7))
            nc.vector.tensor_scalar_add(
                out_t[:, c, tt * 512:(tt + 1) * 512], ps[:], b_t[:, c:c + 1])

        def proj_v(t3):
            ps = psp.tile([128, EL], F32, tag="lg", name=f"vps{t3}")
            for k in range(8):
                nc.tensor.matmul(
                    ps[:],
                    lhsT=xv[:, k, t3 * 128:(t3 + 1) * 128],
                    rhs=wv[:, k, :],
                    start=(k == 0), stop=(k == 7))
            nc.vector.tensor_add(
                VP[:, t3, :].rearrange("p (h x) -> p h x", h=4)[:, :, 0:64],
                ps[:].rearrange("p (h x) -> p h x", h=4),
                bvb[:].rearrange("p (h x) -> p h x", h=4))

        # ---- O-projection for one 128-row output chunk ----
        def oproj_mm(ops, tt, c):
            for eo in range(2):
                nc.tensor.matmul(
                    ops[:, eo, :],
                    lhsT=valsT[:, c, tt * 128:(tt + 1) * 128],
                    rhs=wo[:, c, eo * 512:(eo + 1) * 512],
                    start=(c == 0), stop=(c == 1))

        def oproj_out(ops, tt):
            ot = otp.tile([128, 2, 512], F16, tag="ot", name=f"ot{tt}")
            nc.vector.tensor_copy(ot[:], ops[:])
            nc.sync.dma_start(
                out_d.ap()[tt * 128:(tt + 1) * 128, :]
                .rearrange("p (a b) -> p a b", a=2), ot[:])

        def oproj_tt(tt):
            ops = psp.tile([128, 2, 512], F32, tag="lg", name=f"ops{tt}")
            for c in range(2):
                oproj_mm(ops, tt, c)
            oproj_out(ops, tt)

        # ---- one attention unit: (q-tile qt) x (head-pair chunk c) ----
        def unit(qt, c, opq):
            nkt = 4 * qt + 4
            acc = psp.tile([65, 2, 512], F32, tag="acc", name=f"acc{qt}_{c}")
            exs = {}

            def lg_exp(kt):
                dd = max(0, kt * 128 - qt * 512)
                lg = psp.tile([128, 2, 512], F32, tag="lg",
                              name=f"lg{qt}_{c}_{kt}")
                for hh in range(2):
                    nc.tensor.matmul(
                        lg[:, hh, :],
                        lhsT=KT[hh * 64:(hh + 1) * 64, c,
                                kt * 128:(kt + 1) * 128],
                        rhs=QT[hh * 64:(hh + 1) * 64, c,
                               qt * 512:(qt + 1) * 512],
                        start=True, stop=True)
                ex = expp.tile([128, 2, 512], F16, tag="ex",
                               name=f"ex{qt}_{c}_{kt}")
                # full-width: per-call ACT overhead eats any narrowing gain
                nc.scalar.activation(ex[:, :, :], lg[:, :, :],
                                     AF.Exp, scale=0.125)
                if kt * 128 >= qt * 512:  # diagonal: triangular 128-col mask
                    for hh in range(2):
                        nc.vector.tensor_mul(ex[:, hh, dd:dd + 128],
                                             ex[:, hh, dd:dd + 128], tri[:])
                exs[kt] = (ex, dd)

            def attn_v(kt):
                ex, dd = exs.pop(kt)
                for hh in range(2):
                    h = 2 * c + hh
                    nc.tensor.matmul(
                        acc[:, hh, dd:512],
                        lhsT=VP[:, kt, h * 66:h * 66 + 65],
                        rhs=ex[:, hh, dd:512],
                        start=(kt == 0), stop=(kt == nkt - 1),
                        skip_group_check=True)

            for kt in range(nkt):
                lg_exp(kt)
                if kt >= 2:
                    attn_v(kt - 2)
                if kt in (3, 7) and opq:
                    oproj_tt(opq.pop(0))
            attn_v(max(nkt - 2, 0))
            if nkt >= 2:
                attn_v(nkt - 1)
            while opq:
                oproj_tt(opq.pop(0))

            # normalize straight out of PSUM; runs on DVE/GpSimd while the
            # next unit's attention streams on PE/ACT
            # reciprocal on a DMA-transposed [128, 4] column layout
            # (a single-lane [1, 512] reciprocal costs 4us on DVE); the two
            # hh chains are emitted stage-interleaved so they pipeline, DMA
            # kicks ride the idle GpSimd DGE
            denS, lcol, rcol, rr, bcS = {}, {}, {}, {}, {}
            for hh in range(2):
                denS[hh] = smp.tile([1, 512], F32, tag="denS",
                                    name=f"denS{qt}_{c}_{hh}")
                nc.vector.tensor_copy(denS[hh][:], acc[64:65, hh, :])
            for hh in range(2):
                lcol[hh] = smp.tile([128, 4], F32, tag="lcol",
                                    name=f"lcol{qt}_{c}_{hh}")
                nc.gpsimd.dma_start(
                    lcol[hh][:, :],
                    denS[hh][0:1, :].rearrange("p (a b) -> p a b", a=128))
            for hh in range(2):
                rcol[hh] = smp.tile([128, 4], F32, tag="rcol",
                                    name=f"rcol{qt}_{c}_{hh}")
                nc.vector.reciprocal(rcol[hh][:, :], lcol[hh][:, :])
            for hh in range(2):
                rr[hh] = smp.tile([1, 512], F32, tag="rr",
                                  name=f"rr{qt}_{c}_{hh}")
                nc.gpsimd.dma_start(
                    rr[hh][0:1, :].rearrange("p (a b) -> p a b", a=128),
                    rcol[hh][:, :])
            for hh in range(2):
                bcS[hh] = smp.tile([64, 512], F32, tag="bc",
                                   name=f"bc{qt}_{c}_{hh}")
                nc.gpsimd.partition_broadcast(bcS[hh][:], rr[hh][:])
            for hh in range(2):
                nc.vector.tensor_mul(
                    valsT[hh * 64:(hh + 1) * 64, c,
                          qt * 512:(qt + 1) * 512],
                    acc[0:64, hh, :], bcS[hh][:])

        # ---- schedule ----
        for qt in range(4):
            for cc in range(2):
                proj_kq(xk, wk, bkt, KT, qt, cc, "k")
            for cc in range(2):
                proj_kq(xq, wq, bqt, QT, qt, cc, "q")
            for t3 in range(4 * qt, 4 * qt + 4):
                proj_v(t3)
            for cc in range(2):
                opq = ([4 * (qt - 1) + 2 * cc, 4 * (qt - 1) + 2 * cc + 1]
                       if qt >= 1 else [])
                unit(qt, cc, opq)
        # tail: qt=3 O-proj; c0 halves of tt 12/13 run while the last
        # normalize chain is still in flight (valsT c=0 is already final)
        tail_ops = {}
        for tt in (12, 13):
            tail_ops[tt] = psp.tile([128, 2, 512], F32, tag="lg",
                                    name=f"ops{tt}")
            oproj_mm(tail_ops[tt], tt, 0)
        for tt in (12, 13):
            oproj_mm(tail_ops[tt], tt, 1)
            oproj_out(tail_ops[tt], tt)
        for tt in (14, 15):
            oproj_tt(tt)

    nc.compile()
    return nc


def get_nc():
    if "nc" not in _CACHE:
        _CACHE["nc"] = _build()
    return _CACHE["nc"]


def _tri():
    i = np.arange(128)[:, None]
    j = np.arange(128)[None, :]
    return (i <= j).astype(NP16)


def make_in_maps(query, key, value, Wq, bq, Wk, bk, Wv, bv, Wo, bo):
    query = np.asarray(query, np.float32)
    key = np.asarray(key, np.float32)
    value = np.asarray(value, np.float32)
    Wq, Wk, Wv, Wo = (np.asarray(a, np.float32) for a in (Wq, Wk, Wv, Wo))
    bq, bk, bv = (np.asarray(a, np.float32) for a in (bq, bk, bv))
    tri = _tri()
    vones = np.ones((128, 16, 4, 1), NP16)

    def xlay(x):
        # x [S, E] -> x.T [E, S] -> [chunk 4, partition 128, k 8, 512]
        return np.ascontiguousarray(
            x.T.reshape(8, 128, 4, 512).transpose(2, 1, 0, 3)).astype(NP16)

    def wlay(w):
        # w [e', E] -> w.T [E, e'] -> [partition 128, k 8, e']
        return np.ascontiguousarray(
            w.T.reshape(8, 128, -1).transpose(1, 0, 2)).astype(NP16)

    in_maps = []
    for c in range(NCORES):
        b, g = divmod(c, 4)
        sl = slice(g * EL, (g + 1) * EL)
        in_maps.append({
            "xqT": xlay(query[b]),
            "xkT": xlay(key[b]),
            "xvT": xlay(value[b]),
            "wqT": wlay(Wq[sl, :]),
            "wkT": wlay(Wk[sl, :]),
            "wvT": wlay(Wv[sl, :]),
            "woT": np.ascontiguousarray(
                Wo[:, sl].T.reshape(2, 128, E).transpose(1, 0, 2)
            ).astype(NP16),
            "bq": np.ascontiguousarray(bq[sl]),
            "bk": np.ascontiguousarray(bk[sl]),
            "bv": np.ascontiguousarray(bv[sl]),
            "vones": vones,
            "tri": tri,
        })
    return in_maps


def run(inputs, trace=False, tmpdir=None):
    """Run on 8 cores; returns (full_output, BassKernelResults)."""
    nc = get_nc()
    in_maps = make_in_maps(**inputs)
    res = bass_utils.run_bass_kernel_spmd(
        nc, in_maps, list(range(NCORES)), trace=trace, tmpdir=tmpdir)
    bo = np.asarray(inputs["bo"], np.float32)
    out = np.zeros((B, S, E), np.float32)
    for c in range(NCORES):
        out[c // 4] += res.results[c]["out"].astype(np.float32)
    out += bo[None, None, :]
    return out, res


def kernel(**inputs):
    out, _ = run(inputs)
    return out
